# revision 1
# baseline (speedup 1.0000x reference)
# CCAM channel-attention kernel for Trainium2 (Bass/Tile), 8-core SPMD.
#
# Math (per batch b):
#   q = x[b].reshape(C, N)                      # N = H*W = 4096
#   energy = q @ kbank                          # (C, 64), kbank = martx[0]
#   att = softmax(aphal * (rowmax(energy) - energy), axis=-1)
#   out = gamma * (att @ kbank.T) + x[b]
#
# Sharding: data-parallel over batch B=16 across 8 cores (2 batches/core);
# kbank, aphal, gamma are replicated.  aphal/gamma are baked into the
# program as immediates (cache keyed on their values).
#
# Per-core layout: the 2048 (b,c) rows are processed in 16 tiles of 128
# rows.  The contraction of matmul-1 runs over n, so q must be transposed
# on-chip: 32 PE transposes (fp32) per tile, cast to bf16 during the
# mandatory PSUM->SBUF copy (ScalarE).  Both matmuls run in bf16 (the
# attention output is a small residual correction to x, so bf16 error is
# negligible in the final fp32 output).  Softmax normalization and gamma
# are folded into the fused (psum * (gamma/s)) + x residual op on DVE.

import numpy as np
from contextlib import ExitStack

B, C = 16, 1024
HW = 4096          # H*W
KD = 64            # key bank dim
N_CORES = 8
P = 128            # partitions
ROWS = (B // N_CORES) * C   # 2048 rows per core
NT = ROWS // P              # 16 row tiles per core
NCH = HW // P               # 32 contraction chunks
NF = HW // 512              # 8 output free-dim chunks

_programs = {}


def _build_program(aphal: float, gamma: float, cfg: dict | None = None):
    cfg = cfg or {}
    xs_bufs = cfg.get("xs_bufs", 5)
    qts_bufs = cfg.get("qts_bufs", 2)
    outs_bufs = cfg.get("outs_bufs", 2)
    pst_bufs = cfg.get("pst_bufs", 3)
    pse_bufs = cfg.get("pse_bufs", 1)
    psa_bufs = cfg.get("psa_bufs", 2)
    pso_bufs = cfg.get("pso_bufs", 2)
    split_in = cfg.get("split_in", 1)    # x load split per tile
    split_out = cfg.get("split_out", 1)  # out store split per tile
    dma_only = cfg.get("dma_only", False)  # timing-study mutant: no compute
    prefetch = cfg.get("prefetch", 3)    # x loads emitted this many tiles ahead
    qt_chunk = cfg.get("qt_chunk", 4)    # transposes per PSUM group (4 or 8)
    res_chunk = cfg.get("res_chunk", 4)  # 128-col blocks per residual op (4 or 8)
    import concourse.mybir as mybir
    import concourse.tile as tile
    from concourse import bacc
    from concourse.masks import make_identity

    f32 = mybir.dt.float32
    bf16 = mybir.dt.bfloat16

    nc = bacc.Bacc(
        "TRN2",
        target_bir_lowering=False,
        debug=False,
        enable_asserts=False,
        num_devices=N_CORES,
    )
    x_d = nc.dram_tensor("x", (ROWS, HW), f32, kind="ExternalInput").ap()
    kb_d = nc.dram_tensor("kb", (HW, KD), f32, kind="ExternalInput").ap()
    out_d = nc.dram_tensor("out", (ROWS, HW), f32, kind="ExternalOutput").ap()

    with tile.TileContext(nc) as tc, ExitStack() as ctx:
        const = ctx.enter_context(tc.tile_pool(name="const", bufs=1))
        xs = ctx.enter_context(tc.tile_pool(name="xs", bufs=xs_bufs))
        qts = ctx.enter_context(tc.tile_pool(name="qts", bufs=qts_bufs))
        outs = ctx.enter_context(tc.tile_pool(name="outs", bufs=outs_bufs))
        small = ctx.enter_context(tc.tile_pool(name="small", bufs=6))
        ps_t = ctx.enter_context(tc.tile_pool(name="ps_t", bufs=pst_bufs, space="PSUM"))
        ps_e = ctx.enter_context(tc.tile_pool(name="ps_e", bufs=pse_bufs, space="PSUM"))
        ps_a = ctx.enter_context(tc.tile_pool(name="ps_a", bufs=psa_bufs, space="PSUM"))
        ps_o = ctx.enter_context(tc.tile_pool(name="ps_o", bufs=pso_bufs, space="PSUM"))

        ident32 = const.tile([P, P], f32)
        make_identity(nc, ident32)
        ident16 = const.tile([P, P], bf16)
        make_identity(nc, ident16)

        # kbank in chunked layout: kb_sb[p, a, k] = kbank[a*128 + p, k]
        kb_sb = const.tile([P, NCH, KD], f32)
        nc.sync.dma_start(out=kb_sb, in_=kb_d.rearrange("(a p) k -> p a k", p=P))
        kb16 = const.tile([P, NCH, KD], bf16)
        nc.vector.tensor_copy(kb16, kb_sb)

        # kbank^T in bf16: kbT16[k, n]
        kbT16 = const.tile([KD, HW], bf16)
        for a in range(NCH):
            pst = ps_a.tile([KD, P], f32, tag="psa")
            nc.tensor.transpose(pst, kb_sb[:, a, :], ident32)
            nc.scalar.copy(kbT16[:, a * P:(a + 1) * P], pst)

        xts = {}

        def load_x(t):
            xt = xs.tile([P, NCH, P], f32)
            x_src = x_d[t * P:(t + 1) * P, :].rearrange("p (a q) -> p a q", q=P)
            ci = NCH // split_in
            for s in range(split_in):
                nc.sync.dma_start(
                    out=xt[:, s * ci:(s + 1) * ci, :],
                    in_=x_src[:, s * ci:(s + 1) * ci, :],
                )
            xts[t] = xt

        for t in range(min(prefetch, NT)):
            load_x(t)

        for t in range(NT):
            # --- load x tile (128 rows x 4096) ---
            if t + prefetch < NT:
                load_x(t + prefetch)
            elif t not in xts:
                load_x(t)
            xt = xts.pop(t)

            if dma_only:
                o_dst = out_d[t * P:(t + 1) * P, :].rearrange(
                    "p (a q) -> p a q", q=P
                )
                nc.sync.dma_start(out=o_dst, in_=xt)
                continue

            # --- transpose q: 32 PE transposes, qt_chunk per PSUM group, cast bf16 ---
            qT16 = qts.tile([P, NCH, P], bf16)
            for g in range(NCH // qt_chunk):
                psq = ps_t.tile([P, qt_chunk, P], f32)
                for j in range(qt_chunk):
                    a = qt_chunk * g + j
                    nc.tensor.transpose(psq[:, j, :], xt[:, a, :], ident32)
                nc.scalar.copy(
                    qT16[:, qt_chunk * g:qt_chunk * (g + 1), :], psq
                )

            # --- energy = q @ kbank : accumulate over 32 chunks ---
            pse = ps_e.tile([P, KD], f32)
            for a in range(NCH):
                nc.tensor.matmul(
                    pse,
                    lhsT=qT16[:, a, :],
                    rhs=kb16[:, a, :],
                    start=(a == 0),
                    stop=(a == NCH - 1),
                )

            # --- inverted softmax: exp(aphal*(max - e)), unnormalized ---
            mx = small.tile([P, 1], f32)
            nc.vector.reduce_max(mx, pse, axis=mybir.AxisListType.X)
            mxs = small.tile([P, 1], f32)
            nc.vector.tensor_scalar_mul(mxs, mx, float(aphal))
            att16 = small.tile([P, KD], bf16)
            ssum = small.tile([P, 1], f32)
            nc.scalar.activation(
                att16,
                pse,
                mybir.ActivationFunctionType.Exp,
                bias=mxs,
                scale=-float(aphal),
                accum_out=ssum,
            )
            rinv = small.tile([P, 1], f32)
            nc.vector.reciprocal(rinv, ssum)
            rg = small.tile([P, 1], f32)
            nc.vector.tensor_scalar_mul(rg, rinv, float(gamma))

            # --- att^T (PE transpose, bf16) ---
            psa = ps_a.tile([KD, P], bf16, tag="psa")
            nc.tensor.transpose(psa, att16, ident16)
            attT = small.tile([KD, P], bf16)
            nc.scalar.copy(attT, psa)

            # --- out = (att @ kbank^T) * (gamma/s) + x ;  DMA out ---
            ot = outs.tile([P, NCH, P], f32)
            mm_per_res = res_chunk // 4  # matmuls (N=512) per residual op
            for r in range(NCH // res_chunk):
                pso = ps_o.tile([P, res_chunk, P], f32)
                for m in range(mm_per_res):
                    nf = r * mm_per_res + m
                    nc.tensor.matmul(
                        pso[:, 4 * m:4 * (m + 1), :],
                        lhsT=attT,
                        rhs=kbT16[:, nf * 512:(nf + 1) * 512],
                        start=True,
                        stop=True,
                    )
                nc.vector.scalar_tensor_tensor(
                    out=ot[:, res_chunk * r:res_chunk * (r + 1), :],
                    in0=pso,
                    scalar=rg,
                    in1=xt[:, res_chunk * r:res_chunk * (r + 1), :],
                    op0=mybir.AluOpType.mult,
                    op1=mybir.AluOpType.add,
                )
            o_dst = out_d[t * P:(t + 1) * P, :].rearrange("p (a q) -> p a q", q=P)
            co = NCH // split_out
            for s in range(split_out):
                nc.sync.dma_start(
                    out=o_dst[:, s * co:(s + 1) * co, :],
                    in_=ot[:, s * co:(s + 1) * co, :],
                )

    nc.compile()
    return nc


def _get_program(aphal: float, gamma: float):
    key = (aphal, gamma)
    if key not in _programs:
        _programs[key] = _build_program(aphal, gamma)
    return _programs[key]


def run(x, martx, aphal, gamma, trace=False):
    """Returns (output, BassKernelResults)."""
    from concourse.bass_utils import run_bass_kernel_spmd
    from concourse.bass_interp import get_hw_module

    x = np.ascontiguousarray(np.asarray(x, dtype=np.float32))
    kb = np.ascontiguousarray(
        np.asarray(martx, dtype=np.float32).reshape(HW, KD)
    )
    a_val = float(np.asarray(aphal).reshape(-1)[0])
    g_val = float(np.asarray(gamma).reshape(-1)[0])

    nc = _get_program(a_val, g_val)
    shards = x.reshape(N_CORES, ROWS, HW)
    in_maps = [{"x": shards[i], "kb": kb} for i in range(N_CORES)]

    old_m = nc.m
    nc.m = get_hw_module(nc.m)
    try:
        res = run_bass_kernel_spmd(
            nc, in_maps, core_ids=list(range(N_CORES)), trace=trace
        )
    finally:
        nc.m = old_m

    out = np.stack([res.results[i]["out"] for i in range(N_CORES)])
    out = out.reshape(B, C, 64, 64).astype(np.float32)
    return out, res


def kernel(x, martx, aphal, gamma):
    out, _ = run(x, martx, aphal, gamma, trace=False)
    return out



# revision 8
# speedup vs baseline: 4.1165x; 4.1165x over previous
# CCAM channel-attention kernel for Trainium2 (Bass/Tile), 8-core SPMD.
#
# Math (per batch b):
#   q = x[b].reshape(C, N)                      # N = H*W = 4096
#   energy = q @ kbank                          # (C, 64), kbank = martx[0]
#   att = softmax(aphal * (rowmax(energy) - energy), axis=-1)
#   out = gamma * (att @ kbank.T) + x[b]
#
# Sharding: data-parallel over batch B=16 across 8 cores (2 batches/core);
# kbank, aphal, gamma are replicated.  aphal/gamma are baked into the
# program as immediates (cache keyed on their values).
#
# Per-core layout: the 2048 (b,c) rows are processed in 16 tiles of 128
# rows.  The contraction of matmul-1 runs over n, so q must be transposed
# on-chip: 32 PE transposes (fp32) per tile, cast to bf16 during the
# mandatory PSUM->SBUF copy (ScalarE).  Both matmuls run in bf16 (the
# attention output is a small residual correction to x, so bf16 error is
# negligible in the final fp32 output).  Softmax normalization and gamma
# are folded into the fused (psum * (gamma/s)) + x residual op on DVE.

import numpy as np
from contextlib import ExitStack

B, C = 16, 1024
HW = 4096          # H*W
KD = 64            # key bank dim
N_CORES = 8
P = 128            # partitions
ROWS = (B // N_CORES) * C   # 2048 rows per core
NT = ROWS // P              # 16 row tiles per core
NCH = HW // P               # 32 contraction chunks
NF = HW // 512              # 8 output free-dim chunks

_programs = {}


def _build_program(aphal: float, gamma: float, cfg: dict | None = None):
    cfg = cfg or {}
    xs_bufs = cfg.get("xs_bufs", 4)
    qts_bufs = cfg.get("qts_bufs", 2)
    outs_bufs = cfg.get("outs_bufs", 2)
    pst_bufs = cfg.get("pst_bufs", 3)
    pse_bufs = cfg.get("pse_bufs", 1)
    psa_bufs = cfg.get("psa_bufs", 2)
    pso_bufs = cfg.get("pso_bufs", 2)
    split_in = cfg.get("split_in", 1)    # x load split per super-tile
    split_out = cfg.get("split_out", 1)  # out store split per super-tile
    dma_only = cfg.get("dma_only", False)  # timing-study mutant: no compute
    reps = cfg.get("reps", 1)  # timing-study: repeat identical computation
    merge = cfg.get("merge", 2)  # 128-row tiles per DMA super-tile
    inplace_res = cfg.get("inplace_res", True)  # residual overwrites x tile
    prefetch = cfg.get("prefetch", 3)  # super-tile loads emitted ahead
    store_eng = cfg.get("store_eng", "sync")  # HWDGE ring for out stores
    qt_chunk = cfg.get("qt_chunk", 4)    # transposes per PSUM group (4 or 8)
    res_chunk = cfg.get("res_chunk", 4)  # 128-col blocks per residual op (4 or 8)
    import concourse.mybir as mybir
    import concourse.tile as tile
    from concourse import bacc
    from concourse.masks import make_identity

    f32 = mybir.dt.float32
    bf16 = mybir.dt.bfloat16

    nc = bacc.Bacc(
        "TRN2",
        target_bir_lowering=False,
        debug=False,
        enable_asserts=False,
        num_devices=N_CORES,
    )
    x_d = nc.dram_tensor("x", (ROWS, HW), f32, kind="ExternalInput").ap()
    kb_d = nc.dram_tensor("kb", (HW, KD), f32, kind="ExternalInput").ap()
    out_d = nc.dram_tensor("out", (ROWS, HW), f32, kind="ExternalOutput").ap()

    with tile.TileContext(nc) as tc, ExitStack() as ctx:
        const = ctx.enter_context(tc.tile_pool(name="const", bufs=1))
        xs = ctx.enter_context(tc.tile_pool(name="xs", bufs=xs_bufs))
        qts = ctx.enter_context(tc.tile_pool(name="qts", bufs=qts_bufs))
        outs = ctx.enter_context(tc.tile_pool(name="outs", bufs=outs_bufs))
        small = ctx.enter_context(tc.tile_pool(name="small", bufs=6))
        ps_t = ctx.enter_context(tc.tile_pool(name="ps_t", bufs=pst_bufs, space="PSUM"))
        ps_e = ctx.enter_context(tc.tile_pool(name="ps_e", bufs=pse_bufs, space="PSUM"))
        ps_a = ctx.enter_context(tc.tile_pool(name="ps_a", bufs=psa_bufs, space="PSUM"))
        ps_o = ctx.enter_context(tc.tile_pool(name="ps_o", bufs=pso_bufs, space="PSUM"))

        ident32 = const.tile([P, P], f32)
        make_identity(nc, ident32)
        ident16 = const.tile([P, P], bf16)
        make_identity(nc, ident16)

        # kbank in chunked layout: kb_sb[p, a, k] = kbank[a*128 + p, k]
        kb_sb = const.tile([P, NCH, KD], f32)
        nc.sync.dma_start(out=kb_sb, in_=kb_d.rearrange("(a p) k -> p a k", p=P))
        kb16 = const.tile([P, NCH, KD], bf16)
        nc.vector.tensor_copy(kb16, kb_sb)

        # kbank^T in bf16: kbT16[k, n]
        kbT16 = const.tile([KD, HW], bf16)
        for a in range(NCH):
            pst = ps_a.tile([KD, P], f32, tag="psa")
            nc.tensor.transpose(pst, kb_sb[:, a, :], ident32)
            nc.scalar.copy(kbT16[:, a * P:(a + 1) * P], pst)

        NS = NT // merge            # super-tiles per rep
        SROWS = P * merge           # rows per super-tile
        xts = {}
        iters = [(rep, s) for rep in range(reps) for s in range(NS)]

        def load_x(i):
            s = iters[i][1]
            xt = xs.tile([P, merge, NCH, P], f32)
            x_src = x_d[s * SROWS:(s + 1) * SROWS, :].rearrange(
                "(b p) (a q) -> p b a q", p=P, q=P
            )
            ci = NCH // split_in
            for sp in range(split_in):
                nc.sync.dma_start(
                    out=xt[:, :, sp * ci:(sp + 1) * ci, :],
                    in_=x_src[:, :, sp * ci:(sp + 1) * ci, :],
                )
            xts[i] = xt

        for i in range(min(prefetch, len(iters))):
            load_x(i)

        for i, (rep, s) in enumerate(iters):
            # --- load x super-tile (merge*128 rows x 4096) ---
            if i + prefetch < len(iters):
                load_x(i + prefetch)
            elif i not in xts:
                load_x(i)
            xt = xts.pop(i)
            o_dst = out_d[s * SROWS:(s + 1) * SROWS, :].rearrange(
                "(b p) (a q) -> p b a q", p=P, q=P
            )

            if dma_only:
                nc.sync.dma_start(out=o_dst, in_=xt)
                continue

            ot = xt if inplace_res else outs.tile([P, merge, NCH, P], f32)
            for b in range(merge):
                xtb = xt[:, b]
                # --- transpose q: PE transposes, qt_chunk per PSUM group ---
                qT16 = qts.tile([P, NCH, P], bf16)
                for g in range(NCH // qt_chunk):
                    psq = ps_t.tile([P, qt_chunk, P], f32)
                    for j in range(qt_chunk):
                        a = qt_chunk * g + j
                        nc.tensor.transpose(psq[:, j, :], xtb[:, a, :], ident32)
                    nc.scalar.copy(
                        qT16[:, qt_chunk * g:qt_chunk * (g + 1), :], psq
                    )

                # --- energy = q @ kbank : accumulate over 32 chunks ---
                pse = ps_e.tile([P, KD], f32)
                for a in range(NCH):
                    nc.tensor.matmul(
                        pse,
                        lhsT=qT16[:, a, :],
                        rhs=kb16[:, a, :],
                        start=(a == 0),
                        stop=(a == NCH - 1),
                    )

                # --- inverted softmax: exp(aphal*(max - e)), unnormalized ---
                mx = small.tile([P, 1], f32)
                nc.vector.reduce_max(mx, pse, axis=mybir.AxisListType.X)
                mxs = small.tile([P, 1], f32)
                nc.vector.tensor_scalar_mul(mxs, mx, float(aphal))
                att16 = small.tile([P, KD], bf16)
                ssum = small.tile([P, 1], f32)
                nc.scalar.activation(
                    att16,
                    pse,
                    mybir.ActivationFunctionType.Exp,
                    bias=mxs,
                    scale=-float(aphal),
                    accum_out=ssum,
                )
                rinv = small.tile([P, 1], f32)
                nc.vector.reciprocal(rinv, ssum)
                rg = small.tile([P, 1], f32)
                nc.vector.tensor_scalar_mul(rg, rinv, float(gamma))

                # --- att^T (PE transpose, bf16) ---
                psa = ps_a.tile([KD, P], bf16, tag="psa")
                nc.tensor.transpose(psa, att16, ident16)
                attT = small.tile([KD, P], bf16)
                nc.scalar.copy(attT, psa)

                # --- out = (att @ kbank^T) * (gamma/s) + x ---
                mm_per_res = res_chunk // 4  # matmuls (N=512) per residual op
                for r in range(NCH // res_chunk):
                    pso = ps_o.tile([P, res_chunk, P], f32)
                    for m in range(mm_per_res):
                        nf = r * mm_per_res + m
                        nc.tensor.matmul(
                            pso[:, 4 * m:4 * (m + 1), :],
                            lhsT=attT,
                            rhs=kbT16[:, nf * 512:(nf + 1) * 512],
                            start=True,
                            stop=True,
                        )
                    nc.vector.scalar_tensor_tensor(
                        out=ot[:, b, res_chunk * r:res_chunk * (r + 1), :],
                        in0=pso,
                        scalar=rg,
                        in1=xtb[:, res_chunk * r:res_chunk * (r + 1), :],
                        op0=mybir.AluOpType.mult,
                        op1=mybir.AluOpType.add,
                    )

            co = NCH // split_out
            for sp in range(split_out):
                nc.sync.dma_start(
                    out=o_dst[:, :, sp * co:(sp + 1) * co, :],
                    in_=ot[:, :, sp * co:(sp + 1) * co, :],
                )

    nc.compile()
    return nc


def _get_program(aphal: float, gamma: float):
    key = (aphal, gamma)
    if key not in _programs:
        _programs[key] = _build_program(aphal, gamma)
    return _programs[key]


def _make_fn(nc, n_cores, donate=True):
    """Wrap a prebuilt Bass module as a sharded jitted callable (the same
    lowering ``run_bass_via_pjrt`` uses, minus its per-call concat copies).
    Returns (fn, in_names, out_names, mesh); fn takes the global-shape
    input arrays plus one scratch array per output (donated into the
    outputs when ``donate``)."""
    import jax
    import numpy as jnp_np
    from jax.sharding import Mesh, PartitionSpec
    from jax.experimental.shard_map import shard_map
    import concourse.mybir as mybir
    from concourse.bass2jax import (
        install_neuronx_cc_hook,
        _bass_exec_p,
        partition_id_tensor,
    )

    install_neuronx_cc_hook()

    partition_name = (
        nc.partition_id_tensor.name if nc.partition_id_tensor else None
    )
    in_names, out_names, out_avals = [], [], []
    for alloc in nc.m.functions[0].allocations:
        if not isinstance(alloc, mybir.MemoryLocationSet):
            continue
        name = alloc.memorylocations[0].name
        if alloc.kind == "ExternalInput":
            if name != partition_name:
                in_names.append(name)
        elif alloc.kind == "ExternalOutput":
            out_names.append(name)
            shape = tuple(alloc.tensor_shape)
            dtype = mybir.dt.np(alloc.dtype)
            out_avals.append(jax.core.ShapedArray(shape, dtype))
    n_params = len(in_names)
    all_in = list(in_names) + list(out_names)
    if partition_name is not None:
        all_in.append(partition_name)

    def _body(*args):
        operands = list(args)
        if partition_name is not None:
            operands.append(partition_id_tensor())
        outs = _bass_exec_p.bind(
            *operands,
            out_avals=tuple(out_avals),
            in_names=tuple(all_in),
            out_names=tuple(out_names),
            lowering_input_output_aliases=(),
            sim_require_finite=True,
            sim_require_nnan=True,
            nc=nc,
        )
        return tuple(outs)

    devices = jax.devices()[:n_cores]
    mesh = Mesh(jnp_np.asarray(devices), ("core",))
    n_out = len(out_names)
    in_specs = (PartitionSpec("core"),) * (n_params + n_out)
    fn = jax.jit(
        shard_map(
            _body,
            mesh=mesh,
            in_specs=in_specs,
            out_specs=(PartitionSpec("core"),) * n_out,
            check_rep=False,
        ),
        donate_argnums=(
            tuple(range(n_params, n_params + n_out)) if donate else ()
        ),
        keep_unused=True,
    )
    return fn, in_names, out_names, mesh


_execs = {}


def _get_exec(aphal: float, gamma: float):
    key = (aphal, gamma)
    if key not in _execs:
        from concourse.bass_interp import get_hw_module
        import jax
        from jax.sharding import NamedSharding, PartitionSpec

        nc = _get_program(aphal, gamma)
        nc.m = get_hw_module(nc.m)
        fn, in_names, out_names, mesh = _make_fn(nc, N_CORES, donate=False)
        sh = NamedSharding(mesh, PartitionSpec("core"))
        # the kernel writes every output element, so the (non-donated)
        # scratch operand's contents never matter; allocate it on-device
        # once and reuse across calls
        scratch = jax.device_put(
            np.zeros((N_CORES * ROWS, HW), np.float32), sh
        )
        _execs[key] = (fn, in_names, out_names, sh, scratch)
    return _execs[key]


def kernel(x, martx, aphal, gamma):
    import jax

    x = np.ascontiguousarray(np.asarray(x, dtype=np.float32))
    kb = np.ascontiguousarray(
        np.asarray(martx, dtype=np.float32).reshape(HW, KD)
    )
    a_val = float(np.asarray(aphal).reshape(-1)[0])
    g_val = float(np.asarray(gamma).reshape(-1)[0])

    fn, in_names, out_names, sh, scratch = _get_exec(a_val, g_val)
    vals = {
        "x": x.reshape(N_CORES * ROWS, HW),
        "kb": np.broadcast_to(kb, (N_CORES, HW, KD)).reshape(
            N_CORES * HW, KD
        ),
    }
    args = [jax.device_put(vals[nm], sh) for nm in in_names]
    outs = fn(*args, scratch)
    out = np.asarray(outs[out_names.index("out")])
    return out.reshape(B, C, 64, 64)

